# revision 15
# baseline (speedup 1.0000x reference)
"""Trainium2 Bass kernel for batched windowed DFT, v7: block-DFT GEMM device,
O(output) combine on host.

Frames share their hop-size-512 blocks: X_t[k] = sum_c (-i)^{kc} G_{t+c}[k],
G_j = DFT_2048 of block j. With bins grouped by k mod 4 per 128-row tile and
pair-combined signals A_j = b_j + b_{j+2}, D_j = b_j - b_{j+2} (DFT linearity),
the device computes the pairwise spectra
  S+iT = DFT(A) (classes 0/2),  E+iF = DFT(D) (classes 1/3)
as 64 fp16 matmuls (contraction 512, free dim 510, one PSUM bank per output),
evacuates them with 8 strided copies, and ships the raw partials. The host
finishes with ONE add per output element:
  cls0: X = S[t]+S[t+1] ... cls1: X_re = E[t]+F[t+1], X_im = F[t]-E[t+1], etc.
plus the Hann 3-tap frequency stencil, frames 509..512, bin 512, and the
Hermitian half. Host flops stay O(outputs) - less than the Hermitian copy.
"""
import numpy as np

import concourse.bacc as bacc
import concourse.mybir as mybir
import concourse.tile as tile

F32 = mybir.dt.float32
F16 = mybir.dt.float16
N_CORES = 8
TD = 509          # frames covered by device partials (509..512 host columns)
NC = 510          # variant columns (block pairs j and j+2)
TILE_B0 = [1, 3, 513, 515, 0, 2, 516, 514]
TILE_CLASS = [b % 4 for b in TILE_B0]        # [1, 3, 1, 3, 0, 2, 0, 2]


def build_nc(reps=1):
    nc = bacc.Bacc("TRN2", target_bir_lowering=False, debug=False,
                   num_devices=N_CORES)
    sga_d = nc.dram_tensor("sga", [128, 4 * NC], F16, kind="ExternalInput")
    sgd_d = nc.dram_tensor("sgd", [128, 4 * NC], F16, kind="ExternalInput")
    wt_d = nc.dram_tensor("wt", [128, 8192], F16, kind="ExternalInput")
    outG_d = nc.dram_tensor("outG", [128, 16 * NC], F16, kind="ExternalOutput")

    with tile.TileContext(nc) as tc:
        with (
            tc.tile_pool(name="sigp", bufs=1) as sigp,
            tc.tile_pool(name="wts", bufs=1) as wts,
            tc.tile_pool(name="stg", bufs=1) as stg,
            tc.tile_pool(name="ps", bufs=4, space="PSUM") as ps,
        ):
            VA = sigp.tile([128, 4 * NC], F16, tag="VA")
            VD = sigp.tile([128, 4 * NC], F16, tag="VD")
            W = wts.tile([128, 8192], F16, tag="W")
            stG = stg.tile([128, 16 * NC], F16, tag="stG")

            import contextlib
            rep_cm = tc.For_i(0, reps) if reps > 1 else contextlib.nullcontext()
            with rep_cm:
                nc.sync.dma_start(W[:, 0:128], wt_d.ap()[:, 0:128])
                nc.sync.dma_start(W[:, 128:1024], wt_d.ap()[:, 128:1024])
                # small first chunk so tile 0 starts ASAP, then halves
                nc.scalar.dma_start(VD[:, 0:NC], sgd_d.ap()[:, 0:NC])
                nc.gpsimd.dma_start(VD[:, NC:2 * NC], sgd_d.ap()[:, NC:2 * NC])
                nc.scalar.dma_start(VD[:, 2 * NC:4 * NC], sgd_d.ap()[:, 2 * NC:4 * NC])
                nc.gpsimd.dma_start(VA[:, 0:2 * NC], sga_d.ap()[:, 0:2 * NC])
                nc.scalar.dma_start(VA[:, 2 * NC:4 * NC], sga_d.ap()[:, 2 * NC:4 * NC])
                for c in (1, 2, 4, 6):
                    w = 1024 if c < 2 else 2048
                    nc.sync.dma_start(W[:, c * 1024:c * 1024 + w],
                                      wt_d.ap()[:, c * 1024:c * 1024 + w])

                for T in range(8):
                    var = VD if TILE_CLASS[T] in (1, 3) else VA
                    acc = ps.tile([128, 1024], F32, tag="acc")
                    for a in range(4):
                        for comp in (0, 1):
                            wc0 = ((T * 2 + comp) * 4 + a) * 128
                            nc.tensor.matmul(acc[:, comp * 512:comp * 512 + NC],
                                             W[:, wc0:wc0 + 128],
                                             var[:, a * NC:(a + 1) * NC],
                                             start=(a == 0), stop=(a == 3))
                    g0 = T * 2 * NC
                    if T < 7:
                        # one strided evacuation per tile: [128, 2, 510] <- psum
                        src = acc[:].rearrange("p (c n) -> p c n", c=2)[:, :, 0:NC]
                        dst = stG[:, g0:g0 + 2 * NC].rearrange(
                            "p (c n) -> p c n", c=2)
                        eng = nc.vector.tensor_copy if T % 2 else nc.scalar.copy
                        eng(dst, src)
                    else:
                        # last tile: halve the tail by splitting the evac and
                        # the out-DMA across parallel engine/queue pairs
                        nc.vector.tensor_copy(stG[:, g0:g0 + NC], acc[:, 0:NC])
                        nc.scalar.copy(stG[:, g0 + NC:g0 + 2 * NC],
                                       acc[:, 512:512 + NC])
                        nc.scalar.dma_start(outG_d.ap()[:, g0:g0 + NC],
                                            stG[:, g0:g0 + NC])
                        nc.sync.dma_start(outG_d.ap()[:, g0 + NC:g0 + 2 * NC],
                                          stG[:, g0 + NC:g0 + 2 * NC])
                    if T in (3, 5, 6):
                        q0 = {3: 0, 5: 4, 6: 6}[T] * 2 * NC
                        qcol = slice(q0, (T + 1) * 2 * NC)
                        eng = {3: nc.gpsimd, 5: nc.gpsimd, 6: nc.sync}[T]
                        eng.dma_start(outG_d.ap()[:, qcol], stG[:, qcol])
    nc.compile()
    return nc


def host_prep(x, wsin, wcos):
    """Marshal full inputs into per-core input maps (signal pair-combines and
    fp16 DFT stationaries)."""
    x = np.asarray(x, dtype=np.float32)
    B = x.shape[0]
    xp = np.pad(x, ((0, 0), (1024, 1024)), mode="reflect")
    st = xp.strides
    xb = np.lib.stride_tricks.as_strided(
        xp, (B, 512, 512), (st[0], 512 * st[1], st[1]))   # [b, block j, r]
    A = xb[:, 0:NC] + xb[:, 2:NC + 2]
    D = xb[:, 0:NC] - xb[:, 2:NC + 2]

    def to_dev(M):  # [b, j, r] -> [128, a*NC + j]
        return np.ascontiguousarray(
            M.reshape(B, NC, 4, 128).transpose(0, 3, 2, 1).reshape(B, 128, 4 * NC)
        ).astype(np.float16)

    sga, sgd = to_dev(A), to_dev(D)
    r_ = np.arange(512, dtype=np.float64)
    wt = np.zeros((128, 8192), np.float16)
    for T in range(8):
        kq = (TILE_B0[T] + 4 * np.arange(128)).astype(np.float64)
        ang = 2.0 * np.pi * np.outer(r_, kq) / 2048.0          # [r, q]
        for comp in range(2):
            vals = np.cos(ang) if comp == 0 else -np.sin(ang)
            for a in range(4):
                c0 = ((T * 2 + comp) * 4 + a) * 128
                wt[:, c0:c0 + 128] = vals[a * 128:(a + 1) * 128, :]
    return [{"sga": sga[b], "sgd": sgd[b], "wt": wt} for b in range(B)]


def assemble(results, x, wsin, wcos):
    """Host: pairwise combine, bin 512, frames 509..512, Hann stencil,
    Hermitian half."""
    x = np.asarray(x, dtype=np.float32)
    B = len(results)
    xp = np.pad(x, ((0, 0), (1024, 1024)), mode="reflect")
    st = xp.strides
    XR = np.zeros((B, 1025, 513), np.float32)
    XI = np.zeros((B, 1025, 513), np.float32)
    for b in range(B):
        oG = results[b]["outG"].astype(np.float32).reshape(128, 8, 2, NC)
        for T in range(8):
            cls = TILE_CLASS[T]
            bins = TILE_B0[T] + 4 * np.arange(128)
            P = oG[:, T, 0]   # re-partial: S (cls 0/2) or E (cls 1/3)
            Q = oG[:, T, 1]   # im-partial: T' or F
            if cls == 0:
                xr = P[:, 0:TD] + P[:, 1:TD + 1]; xi = Q[:, 0:TD] + Q[:, 1:TD + 1]
            elif cls == 2:
                xr = P[:, 0:TD] - P[:, 1:TD + 1]; xi = Q[:, 0:TD] - Q[:, 1:TD + 1]
            elif cls == 1:
                xr = P[:, 0:TD] + Q[:, 1:TD + 1]; xi = Q[:, 0:TD] - P[:, 1:TD + 1]
            else:
                xr = P[:, 0:TD] - Q[:, 1:TD + 1]; xi = Q[:, 0:TD] + P[:, 1:TD + 1]
            XR[b, bins, :TD] = xr
            XI[b, bins, :TD] = xi
    fr = np.lib.stride_tricks.as_strided(
        xp, (B, 513, 2048), (st[0], 512 * st[1], st[1]))
    frd = fr[:, :TD]
    XR[:, 512, :TD] = frd[:, :, 0::4].sum(2) - frd[:, :, 2::4].sum(2)
    XI[:, 512, :TD] = -(frd[:, :, 1::4].sum(2) - frd[:, :, 3::4].sum(2))
    kk = np.arange(1025, dtype=np.float64)
    ang = 2.0 * np.pi * np.outer(kk, np.arange(2048, dtype=np.float64)) / 2048.0
    ftail = fr[:, TD:513].astype(np.float64)                 # [B, 4, 2048]
    XR[:, :, TD:513] = np.einsum("btn,kn->bkt", ftail, np.cos(ang)).astype(np.float32)
    XI[:, :, TD:513] = np.einsum("btn,kn->bkt", ftail, -np.sin(ang)).astype(np.float32)
    XRm1 = np.concatenate([XR[:, 1:2], XR[:, :-1]], axis=1)
    XIm1 = np.concatenate([-XI[:, 1:2], XI[:, :-1]], axis=1)
    XRp1 = np.concatenate([XR[:, 1:], XR[:, 1023:1024]], axis=1)
    XIp1 = np.concatenate([XI[:, 1:], -XI[:, 1023:1024]], axis=1)
    WR = 0.5 * XR - 0.25 * (XRm1 + XRp1)
    WI = 0.5 * XI - 0.25 * (XIm1 + XIp1)
    R = np.empty((B, 2048, 513), np.float32)
    I = np.empty((B, 2048, 513), np.float32)
    R[:, :1025] = WR
    I[:, :1025] = WI
    R[:, 1025:] = R[:, 1023:0:-1]
    I[:, 1025:] = -I[:, 1023:0:-1]
    return R, I


class _Runner:
    """Build once, jit once, run many (shard_map over the 8 cores)."""

    def __init__(self, reps=1):
        import jax
        from jax.sharding import Mesh, PartitionSpec
        from jax.experimental.shard_map import shard_map
        from concourse.bass2jax import _bass_exec_p, install_neuronx_cc_hook

        install_neuronx_cc_hook()
        self.jax = jax
        nc = build_nc(reps=reps)
        self.nc = nc
        in_names, out_names, out_avals = [], [], []
        for alloc in nc.m.functions[0].allocations:
            if not isinstance(alloc, mybir.MemoryLocationSet):
                continue
            name = alloc.memorylocations[0].name
            if alloc.kind == "ExternalInput":
                in_names.append(name)
            elif alloc.kind == "ExternalOutput":
                out_names.append(name)
                out_avals.append(jax.core.ShapedArray(
                    tuple(alloc.tensor_shape), mybir.dt.np(alloc.dtype)))
        self.in_names, self.out_names, self.out_avals = in_names, out_names, out_avals
        n_params = len(in_names)
        all_names = in_names + out_names

        def _body(*args):
            outs = _bass_exec_p.bind(
                *args,
                out_avals=tuple(out_avals),
                in_names=tuple(all_names),
                out_names=tuple(out_names),
                lowering_input_output_aliases=(),
                sim_require_finite=True,
                sim_require_nnan=True,
                nc=nc,
            )
            return tuple(outs)

        devices = jax.devices()[:N_CORES]
        mesh = Mesh(np.asarray(devices), ("core",))
        n_outs = len(out_names)
        self._fn = jax.jit(
            shard_map(_body, mesh=mesh,
                      in_specs=(PartitionSpec("core"),) * (n_params + n_outs),
                      out_specs=(PartitionSpec("core"),) * n_outs,
                      check_rep=False),
            keep_unused=True,
        )
        self._zeros = [np.zeros((N_CORES * a.shape[0], *a.shape[1:]), a.dtype)
                       for a in out_avals]

    def prepare(self, in_maps):
        pid = self.nc.partition_id_tensor.name if self.nc.partition_id_tensor else None
        in_maps = [
            dict(m, **({pid: np.array([[c]], dtype=np.uint32)} if pid else {}))
            for c, m in enumerate(in_maps)
        ]
        concat = [np.concatenate([np.asarray(m[name]) for m in in_maps], axis=0)
                  for name in self.in_names]
        self._args = [self.jax.device_put(a) for a in concat + self._zeros]
        self.jax.block_until_ready(self._args)

    def run(self):
        out = self._fn(*self._args)
        self.jax.block_until_ready(out)
        return out

    def results(self, out):
        res = []
        for c in range(N_CORES):
            d = {}
            for i, name in enumerate(self.out_names):
                a = np.asarray(out[i])
                d[name] = a.reshape(N_CORES, *self.out_avals[i].shape)[c]
            res.append(d)
        return res


_RUNNER = None


def kernel(x, wsin, wcos):
    """Full inputs in, full output out: returns (real, -imag) as in reference."""
    global _RUNNER
    if _RUNNER is None:
        _RUNNER = _Runner(reps=1)
    ins = host_prep(x, wsin, wcos)
    _RUNNER.prepare(ins)
    out = _RUNNER.run()
    R, I = assemble(_RUNNER.results(out), x, wsin, wcos)
    return R, I
